# revision 25
# baseline (speedup 1.0000x reference)
"""Self-contained TRN2 Bass kernel for nn_BillehColumn_89670327206508.

kernel(**inputs) -> np.ndarray [4, 900000]

Strategy (v2): neurons (i_rec rows) sharded 8 ways across cores. Synapses
whose z-row is entirely zero are dropped on the host (exact for any input;
~5.4x fewer with the reference's bernoulli(0.05) spikes). Survivors are
packed into a per-partition slot-row layout with data-chosen slots-per-row
S; the z gather runs as bulk InstDMAGatherAnt instructions (NTOK indices
each, 256-byte blocks of 16 zT rows, int16 block index col//16) spread
over 4 SWDGE queues, followed by an on-chip residue extract (16 predicated
copies), w-multiply, slot reduce, and the neuron state update. Overflow
chunks live in an extras region placed FIRST in the token stream and are
scatter-added into 8 per-bucket DRAM accumulators (disjoint tensors so the
RMW DMAs pipeline and overlap the remaining gathers), merged elementwise
before the state phase. No collectives.
"""
import time
import contextlib
import numpy as np
import jax

N, R, D, E, B = 50000, 4, 5, 5000000, 4
NCORES = 8
NR, ND = N * R, N * D
N_PER_CORE = N // NCORES            # 6250
ROWS_PER_CORE = N_PER_CORE * R      # 25000
NPP = 49                            # neurons per partition (49*128=6272 >= 6250)
RPP = NPP * R                       # 196 real slot-rows per partition
NBLK = ND // 16                     # 15625 z blocks of 16 rows (256B)
IREC_ROWS = 128 * RPP               # 25088 (+1 dummy row appended)
CHUNK_CAP = 64                      # max overflow chunks per row (uniqueness)
NTOK = 512                          # tokens per dma_gather instruction
SINGLE_PACKET = False
DT = 1.0


# ---------------------------------------------------------------- host side

def plan_capacity(counts):
    """counts: [NR] filtered count per i_rec row -> (S, TPP, XPP, NCOLS)."""
    core_of_row = np.arange(NR) // ROWS_PER_CORE
    rl_of_row = (np.arange(NR) % ROWS_PER_CORE) % RPP
    bucket_of_row = rl_of_row // 13
    best = None
    for S in (4, 8, 16, 32):
        if counts.max() > S * CHUNK_CAP:
            continue
        over = np.maximum(counts - S, 0)
        chunks = -(-over // S)              # ceil
        # uniform columns per (core, bucket)
        key = core_of_row * 16 + bucket_of_row
        per_bucket = np.bincount(key, weights=chunks.astype(np.float64),
                                 minlength=16 * NCORES)
        ncols = int(max(-(-per_bucket.max() // 128), chunks.max()))
        U = 16 * ncols
        q = 128 // S
        xpp = U
        while (RPP + xpp) % q != 0:
            xpp += 1
        tpp = RPP + xpp
        X = tpp * S
        # calibrated: ~0.59us per token-column, ~2.56us per scatter column
        cost = X + 35 * ncols
        if best is None or cost < best[0]:
            best = (cost, S, tpp, xpp, ncols)
    assert best is not None, "row with too many synapses"
    return best[1], best[2], best[3], best[4]


def pack_shards(rec_w, rec_rows, rec_cols, keep, S, TPP, XPP, NCOLS):
    """Slot-pack filtered synapses per core (extras region first).

    Slot-row t in [0, XPP) is extras column t; slot-row XPP+rl is real row
    (p, rl). Extras columns are grouped in 8 buckets of NCOLS columns; the
    extras in bucket k (target rl in [13k, 13k+13)) scatter-add into the
    bucket's private accumulator acc_k at local row p*13 + rl%13.
    """
    X = TPP * S
    rows_f = rec_rows[keep].astype(np.int64)
    cols_f = rec_cols[keep].astype(np.int64)
    w_f = rec_w[keep].astype(np.float32)
    order = np.argsort(rows_f, kind="stable")
    rows_s = rows_f[order]
    cols_s = cols_f[order]
    w_s = w_f[order]
    nf = rows_s.shape[0]
    counts = np.bincount(rows_s, minlength=NR)
    row_start = np.zeros(NR + 1, np.int64)
    np.cumsum(counts, out=row_start[1:])
    rank = np.arange(nf, dtype=np.int64) - row_start[rows_s]

    shards = []
    for c in range(NCORES):
        r0, r1 = c * ROWS_PER_CORE, (c + 1) * ROWS_PER_CORE
        m = (rows_s >= r0) & (rows_s < r1)
        rr = rows_s[m] - r0
        cc = cols_s[m]
        ww = w_s[m]
        rk = rank[m]
        p = rr // RPP
        rl = rr % RPP
        blk_slot = np.zeros((128, TPP, S), np.int16)
        res_slot = np.zeros((128, TPP, S), np.float32)
        w_slot = np.zeros((128, TPP, S), np.float32)
        extra_base = np.full((128, XPP), 128 * 13, np.int32)  # dummy local row

        base_m = rk < S
        bp, brl, brk = p[base_m], rl[base_m], rk[base_m]
        blk_slot[bp, XPP + brl, brk] = (cc[base_m] // 16).astype(np.int16)
        res_slot[bp, XPP + brl, brk] = (cc[base_m] % 16).astype(np.float32)
        w_slot[bp, XPP + brl, brk] = ww[base_m]

        ov = ~base_m
        if ov.any():
            ov_rl = rl[ov]
            ov_p = p[ov]
            ov_chunk = (rk[ov] - S) // S
            ov_pos = (rk[ov] - S) % S
            bucket = ov_rl // 13
            # order by (bucket, row, chunk); same-row chunks consecutive
            okey = bucket * (ROWS_PER_CORE * CHUNK_CAP) + \
                (ov_p * RPP + ov_rl) * CHUNK_CAP + ov_chunk
            uniq, inv = np.unique(okey, return_inverse=True)
            ub_bucket = uniq // (ROWS_PER_CORE * CHUNK_CAP)
            ub_row = (uniq % (ROWS_PER_CORE * CHUNK_CAP)) // CHUNK_CAP
            ub_p = ub_row // RPP
            ub_rl = ub_row % RPP
            # e_local: index within bucket
            bstart = np.searchsorted(ub_bucket, np.arange(17))
            e_local = np.arange(len(uniq)) - bstart[ub_bucket]
            assert (e_local < 128 * NCOLS).all(), "bucket overflow"
            ep_u = e_local // NCOLS
            ex_u = (ub_bucket * NCOLS + e_local % NCOLS)
            # per-synapse placement
            ep = ep_u[inv]
            ex = ex_u[inv]
            blk_slot[ep, ex, ov_pos] = (cc[ov] // 16).astype(np.int16)
            res_slot[ep, ex, ov_pos] = (cc[ov] % 16).astype(np.float32)
            w_slot[ep, ex, ov_pos] = ww[ov]
            extra_base[ep_u, ex_u] = (ub_p * 13 + ub_rl % 13).astype(np.int32)

        blk2 = blk_slot.reshape(128, X)
        TPQ = NTOK // 128                           # tokens/partition/chunk
        CW = NTOK // 16                             # idx cols per chunk
        A = blk2.reshape(8, 16, X // TPQ, TPQ)      # [p_hi, i, c, q]
        tile16 = A.transpose(2, 1, 3, 0)            # [c, i, q, p_hi]
        idx8 = np.tile(tile16.reshape(X // TPQ, 16, CW), (1, 8, 1)) \
            .transpose(1, 0, 2).reshape(128, 8 * X)
        shards.append(dict(
            idx8=np.ascontiguousarray(idx8),
            res_a=np.ascontiguousarray(res_slot.reshape(128, X)),
            w_a=np.ascontiguousarray(w_slot.reshape(128, X)),
            extra_base=extra_base))
    return shards


def relayout_state(inputs_d, core):
    """Build per-core state-phase arrays in [128, NPP, ...] layout (b innermost)."""
    c = core
    n0 = c * N_PER_CORE
    npad = 128 * NPP                      # 6272
    sl = {}

    def nr_tensor(x):                     # x [B, N*R] -> [128, NPP, R, B]
        v = x.reshape(B, N, R)[:, n0:n0 + N_PER_CORE]          # [B, 6250, R]
        out = np.zeros((B, npad, R), x.dtype)
        out[:, :N_PER_CORE] = v
        return np.ascontiguousarray(out.transpose(1, 2, 0).reshape(128, NPP, R, B))

    def n_tensor(x):                      # x [B, N] -> [128, NPP, B]
        v = x[:, n0:n0 + N_PER_CORE]
        out = np.zeros((B, npad), x.dtype)
        out[:, :N_PER_CORE] = v
        return np.ascontiguousarray(out.transpose(1, 0).reshape(128, NPP, B))

    def pn_tensor(x):                     # x [N] -> [128, NPP]
        out = np.zeros(npad, x.dtype)
        out[:N_PER_CORE] = x[n0:n0 + N_PER_CORE]
        return np.ascontiguousarray(out.reshape(128, NPP))

    def pnr_tensor(x):                    # x [N, W] -> [128, NPP, W]
        W = x.shape[1]
        out = np.zeros((npad, W), x.dtype)
        out[:N_PER_CORE] = x[n0:n0 + N_PER_CORE]
        return np.ascontiguousarray(out.reshape(128, NPP, W))

    sl["inputs"] = nr_tensor(inputs_d["inputs"])
    sl["psc_rise"] = nr_tensor(inputs_d["psc_rise"])
    sl["psc"] = nr_tensor(inputs_d["psc"])
    for k in ["v", "r", "asc_1", "asc_2"]:
        sl[k] = n_tensor(inputs_d[k])
    zb = inputs_d["z_buf"].reshape(B, D, N)[:, :, n0:n0 + N_PER_CORE]
    zpad = np.zeros((B, D, npad), np.float32)
    zpad[:, :, :N_PER_CORE] = zb
    sl["z_slice"] = np.ascontiguousarray(zpad.transpose(2, 1, 0).reshape(128, NPP, D, B))
    for k in ["syn_decay", "psc_initial"]:
        sl[k] = pnr_tensor(inputs_d[k])
    for k in ["t_ref", "v_th", "e_l", "v_reset", "g", "decay", "current_factor",
              "voltage_scale", "voltage_offset"]:
        sl[k] = pn_tensor(inputs_d[k])
    for k in ["asc_amps", "k"]:
        sl[k] = pnr_tensor(inputs_d[k])  # [N,2] -> [128, NPP, 2]
    return sl


def assemble_output(core_outs):
    """core_outs: list of [128, NPP, 72] f32 per core -> full [B, N*(5+2R+D)]."""
    OUT = np.zeros((B, N * 18), np.float32)
    segs = [("new_z", 1, N), ("out_v", 1, N), ("new_r", 1, N), ("asc_1", 1, N),
            ("asc_2", 1, N), ("psc_rise", R, N * R), ("psc", R, N * R), ("z_buf", D, N * D)]
    for c in range(NCORES):
        v = core_outs[c].reshape(128 * NPP, 72)[:N_PER_CORE]   # [6250, 72]
        n0 = c * N_PER_CORE
        off_in = 0
        off_out = 0
        for name, width, glob_w in segs:
            blk = v[:, off_in:off_in + width * B].reshape(N_PER_CORE, width, B)
            if name in ("psc_rise", "psc"):
                tgt = OUT[:, off_out:off_out + glob_w].reshape(B, N, width)
                tgt[:, n0:n0 + N_PER_CORE] = blk.transpose(2, 0, 1)
            elif name == "z_buf":
                tgt = OUT[:, off_out:off_out + glob_w].reshape(B, width, N)
                tgt[:, :, n0:n0 + N_PER_CORE] = blk.transpose(2, 1, 0)
            else:
                tgt = OUT[:, off_out:off_out + glob_w].reshape(B, N)
                tgt[:, n0:n0 + N_PER_CORE] = blk[:, 0].transpose(1, 0)
            off_in += width * B
            off_out += glob_w
    return OUT


# ---------------------------------------------------------------- device side

import concourse.bass as bass
import concourse.tile as tile
from concourse import bacc, mybir

F32 = mybir.dt.float32
I32 = mybir.dt.int32
I16 = mybir.dt.int16
OP = mybir.AluOpType
ACT = mybir.ActivationFunctionType
AX = mybir.AxisListType


def build_program(S, TPP, XPP, NCOLS, reps=1, num_devices=NCORES, parts="gs"):
    X = TPP * S
    NB = X // 128                     # batches of 128 tokens/partition
    G = 128 // S                      # slot-rows per batch
    nc = bacc.Bacc("TRN2", target_bir_lowering=False, debug=False,
                   num_devices=num_devices, num_swdge_queues=4)

    def inp(name, shape, dtype=F32):
        return nc.dram_tensor(name, shape, dtype, kind="ExternalInput").ap()

    ztab = inp("ztab", [NBLK, 64])
    idx8 = inp("idx8", [128, 8 * X], I16)
    res_a = inp("res_a", [128, X])
    w_a = inp("w_a", [128, X])
    extra_base = inp("extra_base", [128, XPP], I32)
    inputs_l = inp("inputs_l", [128, RPP * B])
    psc_rise_l = inp("psc_rise_l", [128, RPP * B])
    psc_l = inp("psc_l", [128, RPP * B])
    z_slice = inp("z_slice", [128, NPP * D * B])
    v_l = inp("v_l", [128, NPP * B])
    r_l = inp("r_l", [128, NPP * B])
    asc1_l = inp("asc1_l", [128, NPP * B])
    asc2_l = inp("asc2_l", [128, NPP * B])
    syn_decay_l = inp("syn_decay_l", [128, RPP])
    psc_initial_l = inp("psc_initial_l", [128, RPP])
    k_l = inp("k_l", [128, NPP * 2])
    asc_amps_l = inp("asc_amps_l", [128, NPP * 2])
    pn = {}
    for name in ["t_ref", "v_th", "e_l", "v_reset", "g", "decay",
                 "current_factor", "voltage_scale", "voltage_offset"]:
        pn[name] = inp(name + "_l", [128, NPP])

    out_t = nc.dram_tensor("out", [128, NPP * 72], F32, kind="ExternalOutput")
    irec_d = nc.dram_tensor("irec_d", [IREC_ROWS + 1, B], F32)
    irec_pr = irec_d.ap()[:IREC_ROWS].rearrange("(p rl) b -> p rl b", p=128)
    acc_d = [nc.dram_tensor(f"acc{k}", [128 * 13 + 1, B], F32) for k in range(16)]

    with tile.TileContext(nc) as tc:
        nc_ = tc.nc
        with contextlib.ExitStack() as ctx:
            pool = ctx.enter_context(tc.tile_pool(name="loop", bufs=2))
            spool = ctx.enter_context(tc.tile_pool(name="state", bufs=1))

            def body():
                if "g" not in parts:
                    out_sb0 = spool.tile([128, NPP * 72], F32, tag="out0")
                    nc_.vector.memset(out_sb0[:], 0.0)
                    nc_.sync.dma_start(out_t.ap()[:], out_sb0[:])
                    return
                extras_sb = spool.tile([128, XPP * B], F32, tag="extras")
                # zero the extras accumulators (incl. dummy row)
                zt = spool.tile([128, 14 * B], F32, tag="zt")
                nc_.vector.memset(zt[:], 0.0)
                for k in range(16):
                    nc_.sync.dma_start(
                        acc_d[k].ap()[:128 * 13].rearrange("(p u) b -> p u b", p=128),
                        zt[:, :13 * B].rearrange("p (u b) -> p u b", b=B))
                    nc_.sync.dma_start(acc_d[k].ap()[128 * 13:128 * 13 + 1, :],
                                       zt[:1, :B])
                eb_t = spool.tile([128, XPP], I32, tag="eb")
                nc_.sync.dma_start(eb_t[:], extra_base)
                scattered = 0
                for bt in range(NB):
                    x0 = bt * 128
                    idx_b = pool.tile([128, 1024], I16, tag="idx")
                    nc_.sync.dma_start(idx_b[:], idx8[:, bt * 1024:(bt + 1) * 1024])
                    w_b = pool.tile([128, 128], F32, tag="w")
                    nc_.sync.dma_start(w_b[:], w_a[:, x0:x0 + 128])
                    res_b = pool.tile([128, 128], F32, tag="res")
                    nc_.sync.dma_start(res_b[:], res_a[:, x0:x0 + 128])
                    blk_b = pool.tile([128, 128 * 64], F32, tag="blk")
                    TPQ = NTOK // 128
                    CW = NTOK // 16
                    for c in range(128 // TPQ):
                        nc_.gpsimd.dma_gather(
                            out_ap=blk_b[:, c * TPQ * 64:(c + 1) * TPQ * 64]
                                .rearrange("p (t e) -> p t e", e=64),
                            in_ap=ztab,
                            idxs_ap=idx_b[:, c * CW:(c + 1) * CW],
                            num_idxs=NTOK, num_idxs_reg=NTOK, elem_size=64,
                            queue_num=c % 4, single_packet=SINGLE_PACKET)
                    if "G" in parts:
                        continue
                    zg = pool.tile([128, 128 * B], F32, tag="zg")
                    zg3 = zg[:].rearrange("p (t b) -> p t b", b=B)
                    blk3 = blk_b[:].rearrange("p (t e) -> p t e", e=64)
                    mask = pool.tile([128, 128], mybir.dt.uint8, tag="mask")
                    for m in range(16):
                        nc_.vector.tensor_scalar(
                            out=mask[:], in0=res_b[:], scalar1=float(m),
                            scalar2=None, op0=OP.is_equal)
                        nc_.vector.copy_predicated(
                            out=zg3,
                            mask=mask[:].unsqueeze(2).to_broadcast([128, 128, B]),
                            data=blk3[:, :, 4 * m:4 * m + 4])
                    nc_.vector.tensor_tensor(
                        out=zg3, in0=zg3,
                        in1=w_b[:].unsqueeze(2).to_broadcast([128, 128, B]),
                        op=OP.mult)
                    if "E" in parts:
                        continue
                    r_t = pool.tile([128, G * B], F32, tag="r")
                    r3 = r_t[:].rearrange("p (g b) -> p g b", b=B)
                    for b in range(B):
                        nc_.vector.tensor_reduce(
                            out=r3[:, :, b],
                            in_=zg[:].rearrange("p (t b) -> p t b", b=B)[:, :, b]
                                .rearrange("p (g s) -> p g s", s=S),
                            axis=AX.X, op=OP.add)
                    rl0 = bt * G
                    if rl0 + G <= XPP:           # all extras
                        nc_.vector.tensor_copy(
                            out=extras_sb[:, rl0 * B:(rl0 + G) * B], in_=r_t[:])
                    elif rl0 >= XPP:             # all real rows
                        nc_.sync.dma_start(
                            irec_pr[:, rl0 - XPP:rl0 - XPP + G, :], r3)
                    else:                        # straddle
                        k = XPP - rl0
                        nc_.vector.tensor_copy(
                            out=extras_sb[:, rl0 * B:XPP * B], in_=r_t[:, :k * B])
                        nc_.sync.dma_start(
                            irec_pr[:, 0:G - k, :], r3[:, k:, :])
                    if "G" in parts or "R" in parts:
                        continue
                    if (bt + 1) * G >= XPP and scattered < 16 * NCOLS:
                        nb_left = max(NB - bt - 1, 1)
                        per = -(-(16 * NCOLS) // nb_left) if bt + 1 < NB \
                            else 16 * NCOLS - scattered
                        for _ in range(min(per, 16 * NCOLS - scattered)):
                            e = scattered
                            scattered += 1
                            ci, k = divmod(e, 16)
                            ex = k * NCOLS + ci
                            nc_.gpsimd.indirect_dma_start(
                                out=acc_d[k].ap()[:],
                                out_offset=bass.IndirectOffsetOnAxis(
                                    ap=eb_t[:, ex:ex + 1], axis=0),
                                in_=extras_sb[:, ex * B:(ex + 1) * B],
                                in_offset=None, compute_op=OP.add)

                if "s" not in parts:
                    out_sb0 = spool.tile([128, NPP * 72], F32, tag="out0")
                    nc_.vector.memset(out_sb0[:], 0.0)
                    nc_.sync.dma_start(out_t.ap()[:], out_sb0[:])
                    return
                # ---- state phase (loads/precompute hoisted) ----
                irec2 = spool.tile([128, RPP * B], F32, tag="irec2")
                nc_.sync.dma_start(irec2[:], irec_pr)
                acc_sb = spool.tile([128, 208 * B], F32, tag="acc_sb")
                for k in range(16):
                    nc_.sync.dma_start(
                        acc_sb[:, k * 13 * B:(k + 1) * 13 * B]
                            .rearrange("p (u b) -> p u b", b=B),
                        acc_d[k].ap()[:128 * 13].rearrange("(p u) b -> p u b", p=128))
                nc_.vector.tensor_tensor(out=irec2[:], in0=irec2[:],
                                         in1=acc_sb[:, :RPP * B], op=OP.add)

                def load(name, ap, sz):
                    t = spool.tile([128, sz], F32, tag=name)
                    nc_.sync.dma_start(t[:], ap)
                    return t

                tin = load("inputs", inputs_l, RPP * B)
                tpr = load("psc_rise", psc_rise_l, RPP * B)
                tps = load("psc", psc_l, RPP * B)
                tz = load("z_slice", z_slice, NPP * D * B)
                tv = load("v", v_l, NPP * B)
                tr = load("r", r_l, NPP * B)
                ta1 = load("asc1", asc1_l, NPP * B)
                ta2 = load("asc2", asc2_l, NPP * B)
                tsd = load("syn_decay", syn_decay_l, RPP)
                tpi = load("psc_initial", psc_initial_l, RPP)
                tk = load("k", k_l, NPP * 2)
                tam = load("asc_amps", asc_amps_l, NPP * 2)
                tp = {k_: load(k_, v_, NPP) for k_, v_ in pn.items()}

                out_sb = spool.tile([128, NPP * 72], F32, tag="out")
                o3 = out_sb[:].rearrange("p (n f) -> p n f", f=72)

                def v4(t):   # [128, RPP*B] tile -> [128, NPP, R, B]
                    return t[:].rearrange("p (n r b) -> p n r b", r=R, b=B)

                def v3(t):   # [128, NPP*B] tile -> [128, NPP, B]
                    return t[:].rearrange("p (n b) -> p n b", b=B)

                def o4(lo, hi):  # out slice [128, NPP, R, B]
                    return o3[:, :, lo:hi].rearrange("p n (r b) -> p n r b", b=B)

                def bc_nr(t):  # [128, RPP] tile -> [128, NPP, R, B] b-broadcast
                    return t[:].rearrange("p (n r) -> p n r", r=R).unsqueeze(3) \
                            .to_broadcast([128, NPP, R, B])

                def bc_n(t):   # [128, NPP] tile -> [128, NPP, B] b-broadcast
                    return t[:].unsqueeze(2).to_broadcast([128, NPP, B])

                tmp = spool.tile([128, RPP * B], F32, tag="tmp")
                tmp2 = spool.tile([128, RPP * B], F32, tag="tmp2")
                tmpn = spool.tile([128, NPP * B], F32, tag="tmpn")
                tmpn2 = spool.tile([128, NPP * B], F32, tag="tmpn2")
                tmpn3 = spool.tile([128, NPP * B], F32, tag="tmpn3")
                tmpn4 = spool.tile([128, NPP * B], F32, tag="tmpn4")
                tpn1 = spool.tile([128, NPP], F32, tag="tpn1")
                tpn2 = spool.tile([128, NPP], F32, tag="tpn2")

                # rec_in = irec + inputs
                nc_.vector.tensor_tensor(out=irec2[:], in0=irec2[:], in1=tin[:], op=OP.add)
                # new_psc_rise = syn_decay*psc_rise + rec_in*psc_initial
                nc_.vector.tensor_tensor(out=v4(tmp), in0=v4(tpr), in1=bc_nr(tsd), op=OP.mult)
                nc_.vector.tensor_tensor(out=v4(tmp2), in0=v4(irec2), in1=bc_nr(tpi), op=OP.mult)
                nc_.vector.tensor_tensor(out=o4(20, 36), in0=v4(tmp), in1=v4(tmp2), op=OP.add)
                # new_psc = syn_decay*(psc + DT*psc_rise)
                nc_.vector.scalar_tensor_tensor(out=tmp[:], in0=tpr[:], scalar=DT,
                                                in1=tps[:], op0=OP.mult, op1=OP.add)
                nc_.vector.tensor_tensor(out=o4(36, 52), in0=v4(tmp), in1=bc_nr(tsd), op=OP.mult)
                # input_current = sum_r psc (old)
                psum_view = tps[:].rearrange("p (n r b) -> p n b r", r=R, b=B)
                nc_.vector.tensor_reduce(out=v3(tmpn), in_=psum_view, axis=AX.X, op=OP.add)
                # prev_z
                pz = tz[:].rearrange("p (n d b) -> p n d b", d=D, b=B)[:, :, 0, :]
                # new_r = relu(r + prev_z*t_ref - DT)   (keep pre-relu copy in tmpn2)
                nc_.vector.tensor_tensor(out=v3(tmpn2), in0=pz, in1=bc_n(tp["t_ref"]), op=OP.mult)
                nc_.vector.tensor_tensor(out=tmpn2[:], in0=tmpn2[:], in1=tr[:], op=OP.add)
                nc_.vector.tensor_scalar(out=tmpn2[:], in0=tmpn2[:], scalar1=-DT,
                                         scalar2=None, op0=OP.add)
                nc_.scalar.activation(out=tmpn2[:], in_=tmpn2[:], func=ACT.Relu)
                nc_.vector.tensor_copy(out=o3[:, :, 8:12], in_=v3(tmpn2))
                # e_i = exp(-DT*sigmoid(k))
                nc_.scalar.activation(out=tk[:], in_=tk[:], func=ACT.Sigmoid)
                nc_.vector.tensor_scalar(out=tk[:], in0=tk[:], scalar1=-DT,
                                         scalar2=None, op0=OP.mult)
                nc_.scalar.activation(out=tk[:], in_=tk[:], func=ACT.Exp)
                k2 = tk[:].rearrange("p (n two) -> p n two", two=2)
                am2 = tam[:].rearrange("p (n two) -> p n two", two=2)
                for idx, (tasc, lo) in enumerate([(ta1, 12), (ta2, 16)]):
                    ei = k2[:, :, idx:idx + 1].to_broadcast([128, NPP, B])
                    ai = am2[:, :, idx:idx + 1].to_broadcast([128, NPP, B])
                    nc_.vector.tensor_tensor(out=v3(tmpn3), in0=v3(tasc), in1=ei, op=OP.mult)
                    nc_.vector.tensor_tensor(out=v3(tmpn4), in0=pz, in1=ai, op=OP.mult)
                    nc_.vector.tensor_tensor(out=o3[:, :, lo:lo + 4], in0=v3(tmpn3),
                                             in1=v3(tmpn4), op=OP.add)
                # c1 = input_current + asc1 + asc2 + g*e_l   (asc old)
                nc_.vector.tensor_tensor(out=tpn1[:], in0=tp["g"][:], in1=tp["e_l"][:], op=OP.mult)
                nc_.vector.tensor_tensor(out=tmpn[:], in0=tmpn[:], in1=ta1[:], op=OP.add)
                nc_.vector.tensor_tensor(out=tmpn[:], in0=tmpn[:], in1=ta2[:], op=OP.add)
                nc_.vector.tensor_tensor(out=v3(tmpn), in0=v3(tmpn), in1=bc_n(tpn1), op=OP.add)
                # reset_current = prev_z*(v_reset - v_th)
                nc_.vector.tensor_tensor(out=tpn2[:], in0=tp["v_reset"][:], in1=tp["v_th"][:],
                                         op=OP.subtract)
                nc_.vector.tensor_tensor(out=v3(tmpn3), in0=pz, in1=bc_n(tpn2), op=OP.mult)
                # new_v = decay*v + current_factor*c1 + reset_current
                nc_.vector.tensor_tensor(out=v3(tmpn), in0=v3(tmpn),
                                         in1=bc_n(tp["current_factor"]), op=OP.mult)
                nc_.vector.tensor_tensor(out=v3(tv), in0=v3(tv), in1=bc_n(tp["decay"]), op=OP.mult)
                nc_.vector.tensor_tensor(out=tmpn[:], in0=tmpn[:], in1=tv[:], op=OP.add)
                nc_.vector.tensor_tensor(out=tmpn[:], in0=tmpn[:], in1=tmpn3[:], op=OP.add)
                # out_v = new_v*vscale + voffset
                nc_.vector.tensor_tensor(out=v3(tmpn3), in0=v3(tmpn),
                                         in1=bc_n(tp["voltage_scale"]), op=OP.mult)
                nc_.vector.tensor_tensor(out=o3[:, :, 4:8], in0=v3(tmpn3),
                                         in1=bc_n(tp["voltage_offset"]), op=OP.add)
                # v_sc = (new_v - v_th) / (v_th - e_l)
                nc_.vector.tensor_tensor(out=tpn1[:], in0=tp["v_th"][:], in1=tp["e_l"][:],
                                         op=OP.subtract)
                nc_.vector.reciprocal(out=tpn1[:], in_=tpn1[:])
                nc_.vector.tensor_tensor(out=v3(tmpn), in0=v3(tmpn), in1=bc_n(tp["v_th"]),
                                         op=OP.subtract)
                nc_.vector.tensor_tensor(out=v3(tmpn), in0=v3(tmpn), in1=bc_n(tpn1), op=OP.mult)
                # new_z = (v_sc > 0) & (new_r <= 0)
                nc_.vector.tensor_scalar(out=tmpn[:], in0=tmpn[:], scalar1=0.0, scalar2=None,
                                         op0=OP.is_gt)
                nc_.vector.tensor_scalar(out=tmpn2[:], in0=tmpn2[:], scalar1=0.0, scalar2=None,
                                         op0=OP.is_le)
                nc_.vector.tensor_tensor(out=tmpn[:], in0=tmpn[:], in1=tmpn2[:], op=OP.mult)
                nc_.vector.tensor_copy(out=o3[:, :, 0:4], in_=v3(tmpn))
                # z_buf out
                nc_.vector.tensor_copy(out=o3[:, :, 52:56], in_=v3(tmpn))
                zsrc = tz[:].rearrange("p (n x) -> p n x", x=D * B)[:, :, 0:(D - 1) * B]
                nc_.vector.tensor_copy(out=o3[:, :, 56:72], in_=zsrc)

                nc_.sync.dma_start(out_t.ap()[:], out_sb[:])

            if reps == 1:
                body()
            else:
                with tc.For_i(0, reps, 1):
                    body()

    nc.compile()
    return nc


# ---------------------------------------------------------------- jax runner

from concourse import bass2jax
from concourse.bass2jax import _bass_exec_p, install_neuronx_cc_hook, partition_id_tensor
from jax.sharding import Mesh, PartitionSpec
from jax.experimental.shard_map import shard_map


def make_runner(nc, n_cores):
    install_neuronx_cc_hook()
    assert nc.dbg_addr is None or not nc.dbg_callbacks
    partition_name = nc.partition_id_tensor.name if nc.partition_id_tensor else None
    in_names, out_names, out_avals, zero_outs = [], [], [], []
    for alloc in nc.m.functions[0].allocations:
        if not isinstance(alloc, mybir.MemoryLocationSet):
            continue
        name = alloc.memorylocations[0].name
        if alloc.kind == "ExternalInput":
            if name != partition_name and (nc.dbg_addr is None or name != nc.dbg_addr.name):
                in_names.append(name)
        elif alloc.kind == "ExternalOutput":
            shape = tuple(alloc.tensor_shape)
            dtype = mybir.dt.np(alloc.dtype)
            out_names.append(name)
            out_avals.append(jax.core.ShapedArray(shape, dtype))
            zero_outs.append(np.zeros(shape, dtype))
    n_params = len(in_names)
    n_outs = len(out_avals)
    in_names_all = list(in_names) + list(out_names)
    if partition_name is not None:
        in_names_all.append(partition_name)

    donate = tuple(range(n_params, n_params + n_outs))

    def _body(*args):
        operands = list(args)
        if partition_name is not None:
            operands.append(partition_id_tensor())
        outs = _bass_exec_p.bind(
            *operands, out_avals=tuple(out_avals), in_names=tuple(in_names_all),
            out_names=tuple(out_names), lowering_input_output_aliases=(),
            sim_require_finite=True, sim_require_nnan=True, nc=nc)
        return tuple(outs)

    if n_cores == 1:
        fn = jax.jit(_body, donate_argnums=donate, keep_unused=True)

        def run(in_map):
            args = [np.asarray(in_map[n]) for n in in_names] + [z.copy() for z in zero_outs]
            outs = fn(*args)
            jax.block_until_ready(outs)
            return {name: np.asarray(outs[i]) for i, name in enumerate(out_names)}
        return run

    devices = jax.devices()[:n_cores]
    mesh = Mesh(np.asarray(devices), ("core",))
    fn = jax.jit(
        shard_map(_body, mesh=mesh, in_specs=(PartitionSpec("core"),) * (n_params + n_outs),
                  out_specs=(PartitionSpec("core"),) * n_outs, check_rep=False),
        donate_argnums=donate, keep_unused=True)

    def run(in_maps):
        concat_in = [np.concatenate([np.asarray(m[n]) for m in in_maps], axis=0) for n in in_names]
        concat_zeros = [np.zeros((n_cores * z.shape[0], *z.shape[1:]), z.dtype) for z in zero_outs]
        outs = fn(*concat_in, *concat_zeros)
        jax.block_until_ready(outs)
        return [
            {name: np.asarray(outs[i]).reshape(n_cores, *out_avals[i].shape)[c]
             for i, name in enumerate(out_names)}
            for c in range(n_cores)
        ]
    return run


_CACHE = {}


def get_program(S, TPP, XPP, NCOLS, reps=1, parts="gs"):
    key = (S, TPP, XPP, NCOLS, reps, parts)
    if key not in _CACHE:
        nc = build_program(S, TPP, XPP, NCOLS, reps=reps, parts=parts)
        _CACHE[key] = make_runner(nc, NCORES)
    return _CACHE[key]


def build_in_maps(inputs):
    zT = np.ascontiguousarray(inputs["z_buf"].T)        # [ND, B]
    nonzero_row = (zT != 0.0).any(axis=1)
    keep = nonzero_row[inputs["rec_cols"]]
    counts = np.bincount(inputs["rec_rows"][keep], minlength=NR)
    S, TPP, XPP, NCOLS = plan_capacity(counts)
    shards = pack_shards(inputs["rec_w"], inputs["rec_rows"].astype(np.int64),
                         inputs["rec_cols"].astype(np.int64), keep, S, TPP, XPP,
                         NCOLS)
    ztab = np.ascontiguousarray(zT.reshape(NBLK, 64))
    in_maps = []
    for c in range(NCORES):
        sh = shards[c]
        sl = relayout_state(inputs, c)
        m = dict(ztab=ztab, idx8=sh["idx8"], res_a=sh["res_a"], w_a=sh["w_a"],
                 extra_base=sh["extra_base"])
        m["inputs_l"] = sl["inputs"].reshape(128, -1)
        m["psc_rise_l"] = sl["psc_rise"].reshape(128, -1)
        m["psc_l"] = sl["psc"].reshape(128, -1)
        m["z_slice"] = sl["z_slice"].reshape(128, -1)
        m["v_l"] = sl["v"].reshape(128, -1)
        m["r_l"] = sl["r"].reshape(128, -1)
        m["asc1_l"] = sl["asc_1"].reshape(128, -1)
        m["asc2_l"] = sl["asc_2"].reshape(128, -1)
        m["syn_decay_l"] = sl["syn_decay"].reshape(128, -1)
        m["psc_initial_l"] = sl["psc_initial"].reshape(128, -1)
        m["k_l"] = sl["k"].reshape(128, -1)
        m["asc_amps_l"] = sl["asc_amps"].reshape(128, -1)
        for name in ["t_ref", "v_th", "e_l", "v_reset", "g", "decay",
                     "current_factor", "voltage_scale", "voltage_offset"]:
            m[name + "_l"] = sl[name].reshape(128, -1)
        in_maps.append(m)
    return in_maps, (S, TPP, XPP, NCOLS)


def kernel(**inputs) -> np.ndarray:
    inputs = {k: np.asarray(v) for k, v in inputs.items()}
    in_maps, (S, TPP, XPP, NCOLS) = build_in_maps(inputs)
    run = get_program(S, TPP, XPP, NCOLS, reps=1)
    results = run(in_maps)
    core_outs = [results[c]["out"].reshape(128, NPP, 72) for c in range(NCORES)]
    return assemble_output(core_outs)


# revision 26
# speedup vs baseline: 2.0830x; 2.0830x over previous
"""Self-contained TRN2 Bass kernel for nn_BillehColumn_89670327206508.

kernel(**inputs) -> np.ndarray [4, 900000]

Strategy (v2): neurons (i_rec rows) sharded 8 ways across cores. Synapses
whose z-row is entirely zero are dropped on the host (exact for any input;
~5.4x fewer with the reference's bernoulli(0.05) spikes). Survivors are
packed into a per-partition slot-row layout with data-chosen slots-per-row
S; the z gather runs as bulk InstDMAGatherAnt instructions (NTOK indices
each, 256-byte blocks of 16 zT rows, int16 block index col//16) spread
over 4 SWDGE queues, followed by an on-chip residue extract (16 predicated
copies), w-multiply, slot reduce, and the neuron state update. Overflow
chunks live in an extras region placed FIRST in the token stream and are
scatter-added into 8 per-bucket DRAM accumulators (disjoint tensors so the
RMW DMAs pipeline and overlap the remaining gathers), merged elementwise
before the state phase. No collectives.
"""
import time
import contextlib
import numpy as np
import jax

N, R, D, E, B = 50000, 4, 5, 5000000, 4
NCORES = 8
NR, ND = N * R, N * D
N_PER_CORE = N // NCORES            # 6250
ROWS_PER_CORE = N_PER_CORE * R      # 25000
NPP = 49                            # neurons per partition (49*128=6272 >= 6250)
RPP = NPP * R                       # 196 real slot-rows per partition
NBLK = ND // 16                     # 15625 z blocks of 16 rows (256B)
IREC_ROWS = 128 * RPP               # 25088 (+1 dummy row appended)
CHUNK_CAP = 64                      # max overflow chunks per row (uniqueness)
NTOK = 512                          # tokens per dma_gather instruction
SINGLE_PACKET = False
DT = 1.0


# ---------------------------------------------------------------- host side

def plan_capacity(counts):
    """counts: [NR] filtered count per i_rec row -> (S, TPP, XPP, NCOLS)."""
    core_of_row = np.arange(NR) // ROWS_PER_CORE
    rl_of_row = (np.arange(NR) % ROWS_PER_CORE) % RPP
    bucket_of_row = rl_of_row // 25
    best = None
    for S in (4, 8, 16, 32):
        if counts.max() > S * CHUNK_CAP:
            continue
        over = np.maximum(counts - S, 0)
        chunks = -(-over // S)              # ceil
        # uniform columns per (core, bucket)
        key = core_of_row * 8 + bucket_of_row
        per_bucket = np.bincount(key, weights=chunks.astype(np.float64),
                                 minlength=8 * NCORES)
        ncols = int(max(-(-per_bucket.max() // 128), chunks.max()))
        U = 8 * ncols
        q = 128 // S
        xpp = U
        while (RPP + xpp) % q != 0:
            xpp += 1
        tpp = RPP + xpp
        X = tpp * S
        # calibrated: ~0.59us per token-column, ~2.56us per scatter column
        cost = X + 35 * ncols
        if best is None or cost < best[0]:
            best = (cost, S, tpp, xpp, ncols)
    assert best is not None, "row with too many synapses"
    return best[1], best[2], best[3], best[4]


def pack_shards(rec_w, rec_rows, rec_cols, keep, S, TPP, XPP, NCOLS):
    """Slot-pack filtered synapses per core (extras region first).

    Slot-row t in [0, XPP) is extras column t; slot-row XPP+rl is real row
    (p, rl). Extras columns are grouped in 8 buckets of NCOLS columns; the
    extras in bucket k (target rl in [25k, 25k+25)) scatter-add into the
    bucket's private accumulator acc_k at local row p*25 + rl%25.
    """
    X = TPP * S
    rows_f = rec_rows[keep].astype(np.int64)
    cols_f = rec_cols[keep].astype(np.int64)
    w_f = rec_w[keep].astype(np.float32)
    order = np.argsort(rows_f, kind="stable")
    rows_s = rows_f[order]
    cols_s = cols_f[order]
    w_s = w_f[order]
    nf = rows_s.shape[0]
    counts = np.bincount(rows_s, minlength=NR)
    row_start = np.zeros(NR + 1, np.int64)
    np.cumsum(counts, out=row_start[1:])
    rank = np.arange(nf, dtype=np.int64) - row_start[rows_s]

    shards = []
    for c in range(NCORES):
        r0, r1 = c * ROWS_PER_CORE, (c + 1) * ROWS_PER_CORE
        m = (rows_s >= r0) & (rows_s < r1)
        rr = rows_s[m] - r0
        cc = cols_s[m]
        ww = w_s[m]
        rk = rank[m]
        p = rr // RPP
        rl = rr % RPP
        blk_slot = np.zeros((128, TPP, S), np.int16)
        res_slot = np.zeros((128, TPP, S), np.float32)
        w_slot = np.zeros((128, TPP, S), np.float32)
        extra_base = np.full((128, XPP), 128 * 25, np.int32)  # dummy local row

        base_m = rk < S
        bp, brl, brk = p[base_m], rl[base_m], rk[base_m]
        blk_slot[bp, XPP + brl, brk] = (cc[base_m] // 16).astype(np.int16)
        res_slot[bp, XPP + brl, brk] = (cc[base_m] % 16).astype(np.float32)
        w_slot[bp, XPP + brl, brk] = ww[base_m]

        ov = ~base_m
        if ov.any():
            ov_rl = rl[ov]
            ov_p = p[ov]
            ov_chunk = (rk[ov] - S) // S
            ov_pos = (rk[ov] - S) % S
            bucket = ov_rl // 25
            # order by (bucket, row, chunk); same-row chunks consecutive
            okey = bucket * (ROWS_PER_CORE * CHUNK_CAP) + \
                (ov_p * RPP + ov_rl) * CHUNK_CAP + ov_chunk
            uniq, inv = np.unique(okey, return_inverse=True)
            ub_bucket = uniq // (ROWS_PER_CORE * CHUNK_CAP)
            ub_row = (uniq % (ROWS_PER_CORE * CHUNK_CAP)) // CHUNK_CAP
            ub_p = ub_row // RPP
            ub_rl = ub_row % RPP
            # e_local: index within bucket
            bstart = np.searchsorted(ub_bucket, np.arange(9))
            e_local = np.arange(len(uniq)) - bstart[ub_bucket]
            assert (e_local < 128 * NCOLS).all(), "bucket overflow"
            ep_u = e_local // NCOLS
            ex_u = (ub_bucket * NCOLS + e_local % NCOLS)
            # per-synapse placement
            ep = ep_u[inv]
            ex = ex_u[inv]
            blk_slot[ep, ex, ov_pos] = (cc[ov] // 16).astype(np.int16)
            res_slot[ep, ex, ov_pos] = (cc[ov] % 16).astype(np.float32)
            w_slot[ep, ex, ov_pos] = ww[ov]
            extra_base[ep_u, ex_u] = (ub_p * 25 + ub_rl % 25).astype(np.int32)

        blk2 = blk_slot.reshape(128, X)
        TPQ = NTOK // 128                           # tokens/partition/chunk
        CW = NTOK // 16                             # idx cols per chunk
        A = blk2.reshape(8, 16, X // TPQ, TPQ)      # [p_hi, i, c, q]
        tile16 = A.transpose(2, 1, 3, 0)            # [c, i, q, p_hi]
        idx8 = np.tile(tile16.reshape(X // TPQ, 16, CW), (1, 8, 1)) \
            .transpose(1, 0, 2).reshape(128, 8 * X)
        shards.append(dict(
            idx8=np.ascontiguousarray(idx8),
            res_a=np.ascontiguousarray(res_slot.reshape(128, X)),
            w_a=np.ascontiguousarray(w_slot.reshape(128, X)),
            extra_base=extra_base))
    return shards


def relayout_state(inputs_d, core):
    """Build per-core state-phase arrays in [128, NPP, ...] layout (b innermost)."""
    c = core
    n0 = c * N_PER_CORE
    npad = 128 * NPP                      # 6272
    sl = {}

    def nr_tensor(x):                     # x [B, N*R] -> [128, NPP, R, B]
        v = x.reshape(B, N, R)[:, n0:n0 + N_PER_CORE]          # [B, 6250, R]
        out = np.zeros((B, npad, R), x.dtype)
        out[:, :N_PER_CORE] = v
        return np.ascontiguousarray(out.transpose(1, 2, 0).reshape(128, NPP, R, B))

    def n_tensor(x):                      # x [B, N] -> [128, NPP, B]
        v = x[:, n0:n0 + N_PER_CORE]
        out = np.zeros((B, npad), x.dtype)
        out[:, :N_PER_CORE] = v
        return np.ascontiguousarray(out.transpose(1, 0).reshape(128, NPP, B))

    def pn_tensor(x):                     # x [N] -> [128, NPP]
        out = np.zeros(npad, x.dtype)
        out[:N_PER_CORE] = x[n0:n0 + N_PER_CORE]
        return np.ascontiguousarray(out.reshape(128, NPP))

    def pnr_tensor(x):                    # x [N, W] -> [128, NPP, W]
        W = x.shape[1]
        out = np.zeros((npad, W), x.dtype)
        out[:N_PER_CORE] = x[n0:n0 + N_PER_CORE]
        return np.ascontiguousarray(out.reshape(128, NPP, W))

    sl["inputs"] = nr_tensor(inputs_d["inputs"])
    sl["psc_rise"] = nr_tensor(inputs_d["psc_rise"])
    sl["psc"] = nr_tensor(inputs_d["psc"])
    for k in ["v", "r", "asc_1", "asc_2"]:
        sl[k] = n_tensor(inputs_d[k])
    zb = inputs_d["z_buf"].reshape(B, D, N)[:, :, n0:n0 + N_PER_CORE]
    zpad = np.zeros((B, D, npad), np.float32)
    zpad[:, :, :N_PER_CORE] = zb
    sl["z_slice"] = np.ascontiguousarray(zpad.transpose(2, 1, 0).reshape(128, NPP, D, B))
    for k in ["syn_decay", "psc_initial"]:
        sl[k] = pnr_tensor(inputs_d[k])
    for k in ["t_ref", "v_th", "e_l", "v_reset", "g", "decay", "current_factor",
              "voltage_scale", "voltage_offset"]:
        sl[k] = pn_tensor(inputs_d[k])
    for k in ["asc_amps", "k"]:
        sl[k] = pnr_tensor(inputs_d[k])  # [N,2] -> [128, NPP, 2]
    return sl


def assemble_output(core_outs):
    """core_outs: list of [128, NPP, 72] f32 per core -> full [B, N*(5+2R+D)]."""
    OUT = np.zeros((B, N * 18), np.float32)
    segs = [("new_z", 1, N), ("out_v", 1, N), ("new_r", 1, N), ("asc_1", 1, N),
            ("asc_2", 1, N), ("psc_rise", R, N * R), ("psc", R, N * R), ("z_buf", D, N * D)]
    for c in range(NCORES):
        v = core_outs[c].reshape(128 * NPP, 72)[:N_PER_CORE]   # [6250, 72]
        n0 = c * N_PER_CORE
        off_in = 0
        off_out = 0
        for name, width, glob_w in segs:
            blk = v[:, off_in:off_in + width * B].reshape(N_PER_CORE, width, B)
            if name in ("psc_rise", "psc"):
                tgt = OUT[:, off_out:off_out + glob_w].reshape(B, N, width)
                tgt[:, n0:n0 + N_PER_CORE] = blk.transpose(2, 0, 1)
            elif name == "z_buf":
                tgt = OUT[:, off_out:off_out + glob_w].reshape(B, width, N)
                tgt[:, :, n0:n0 + N_PER_CORE] = blk.transpose(2, 1, 0)
            else:
                tgt = OUT[:, off_out:off_out + glob_w].reshape(B, N)
                tgt[:, n0:n0 + N_PER_CORE] = blk[:, 0].transpose(1, 0)
            off_in += width * B
            off_out += glob_w
    return OUT


# ---------------------------------------------------------------- device side

import concourse.bass as bass
import concourse.tile as tile
from concourse import bacc, mybir

F32 = mybir.dt.float32
I32 = mybir.dt.int32
I16 = mybir.dt.int16
OP = mybir.AluOpType
ACT = mybir.ActivationFunctionType
AX = mybir.AxisListType


def build_program(S, TPP, XPP, NCOLS, reps=1, num_devices=NCORES, parts="gs"):
    X = TPP * S
    NB = X // 128                     # batches of 128 tokens/partition
    G = 128 // S                      # slot-rows per batch
    nc = bacc.Bacc("TRN2", target_bir_lowering=False, debug=False,
                   num_devices=num_devices, num_swdge_queues=4)

    def inp(name, shape, dtype=F32):
        return nc.dram_tensor(name, shape, dtype, kind="ExternalInput").ap()

    ztab = inp("ztab", [NBLK, 64])
    idx8 = inp("idx8", [128, 8 * X], I16)
    res_a = inp("res_a", [128, X])
    w_a = inp("w_a", [128, X])
    extra_base = inp("extra_base", [128, XPP], I32)
    inputs_l = inp("inputs_l", [128, RPP * B])
    psc_rise_l = inp("psc_rise_l", [128, RPP * B])
    psc_l = inp("psc_l", [128, RPP * B])
    z_slice = inp("z_slice", [128, NPP * D * B])
    v_l = inp("v_l", [128, NPP * B])
    r_l = inp("r_l", [128, NPP * B])
    asc1_l = inp("asc1_l", [128, NPP * B])
    asc2_l = inp("asc2_l", [128, NPP * B])
    syn_decay_l = inp("syn_decay_l", [128, RPP])
    psc_initial_l = inp("psc_initial_l", [128, RPP])
    k_l = inp("k_l", [128, NPP * 2])
    asc_amps_l = inp("asc_amps_l", [128, NPP * 2])
    pn = {}
    for name in ["t_ref", "v_th", "e_l", "v_reset", "g", "decay",
                 "current_factor", "voltage_scale", "voltage_offset"]:
        pn[name] = inp(name + "_l", [128, NPP])

    out_t = nc.dram_tensor("out", [128, NPP * 72], F32, kind="ExternalOutput")
    irec_d = nc.dram_tensor("irec_d", [IREC_ROWS + 1, B], F32)
    irec_pr = irec_d.ap()[:IREC_ROWS].rearrange("(p rl) b -> p rl b", p=128)
    acc_d = [nc.dram_tensor(f"acc{k}", [128 * 25 + 1, B], F32) for k in range(8)]

    with tile.TileContext(nc) as tc:
        nc_ = tc.nc
        with contextlib.ExitStack() as ctx:
            pool = ctx.enter_context(tc.tile_pool(name="loop", bufs=2))
            spool = ctx.enter_context(tc.tile_pool(name="state", bufs=1))

            def body():
                if "g" not in parts:
                    out_sb0 = spool.tile([128, NPP * 72], F32, tag="out0")
                    nc_.vector.memset(out_sb0[:], 0.0)
                    nc_.sync.dma_start(out_t.ap()[:], out_sb0[:])
                    return
                extras_sb = spool.tile([128, XPP * B], F32, tag="extras")
                # zero the extras accumulators (incl. dummy row)
                zt = spool.tile([128, 26 * B], F32, tag="zt")
                nc_.vector.memset(zt[:], 0.0)
                for k in range(8):
                    nc_.sync.dma_start(
                        acc_d[k].ap()[:128 * 25].rearrange("(p u) b -> p u b", p=128),
                        zt[:, :25 * B].rearrange("p (u b) -> p u b", b=B))
                    nc_.sync.dma_start(acc_d[k].ap()[128 * 25:128 * 25 + 1, :],
                                       zt[:1, :B])
                eb_t = spool.tile([128, XPP], I32, tag="eb")
                nc_.sync.dma_start(eb_t[:], extra_base)
                scattered = 0
                for bt in range(NB):
                    x0 = bt * 128
                    idx_b = pool.tile([128, 1024], I16, tag="idx")
                    nc_.sync.dma_start(idx_b[:], idx8[:, bt * 1024:(bt + 1) * 1024])
                    w_b = pool.tile([128, 128], F32, tag="w")
                    nc_.sync.dma_start(w_b[:], w_a[:, x0:x0 + 128])
                    res_b = pool.tile([128, 128], F32, tag="res")
                    nc_.sync.dma_start(res_b[:], res_a[:, x0:x0 + 128])
                    blk_b = pool.tile([128, 128 * 64], F32, tag="blk")
                    TPQ = NTOK // 128
                    CW = NTOK // 16
                    for c in range(128 // TPQ):
                        nc_.gpsimd.dma_gather(
                            out_ap=blk_b[:, c * TPQ * 64:(c + 1) * TPQ * 64]
                                .rearrange("p (t e) -> p t e", e=64),
                            in_ap=ztab,
                            idxs_ap=idx_b[:, c * CW:(c + 1) * CW],
                            num_idxs=NTOK, num_idxs_reg=NTOK, elem_size=64,
                            queue_num=c % 4, single_packet=SINGLE_PACKET)
                    if "G" in parts:
                        continue
                    zg = pool.tile([128, 128 * B], F32, tag="zg")
                    zg3 = zg[:].rearrange("p (t b) -> p t b", b=B)
                    blk3 = blk_b[:].rearrange("p (t e) -> p t e", e=64)
                    mask = pool.tile([128, 128], mybir.dt.uint8, tag="mask")
                    for m in range(16):
                        nc_.vector.tensor_scalar(
                            out=mask[:], in0=res_b[:], scalar1=float(m),
                            scalar2=None, op0=OP.is_equal)
                        nc_.vector.copy_predicated(
                            out=zg3,
                            mask=mask[:].unsqueeze(2).to_broadcast([128, 128, B]),
                            data=blk3[:, :, 4 * m:4 * m + 4])
                    nc_.vector.tensor_tensor(
                        out=zg3, in0=zg3,
                        in1=w_b[:].unsqueeze(2).to_broadcast([128, 128, B]),
                        op=OP.mult)
                    if "E" in parts:
                        continue
                    r_t = pool.tile([128, G * B], F32, tag="r")
                    r3 = r_t[:].rearrange("p (g b) -> p g b", b=B)
                    for b in range(B):
                        nc_.vector.tensor_reduce(
                            out=r3[:, :, b],
                            in_=zg[:].rearrange("p (t b) -> p t b", b=B)[:, :, b]
                                .rearrange("p (g s) -> p g s", s=S),
                            axis=AX.X, op=OP.add)
                    rl0 = bt * G
                    if rl0 + G <= XPP:           # all extras
                        nc_.vector.tensor_copy(
                            out=extras_sb[:, rl0 * B:(rl0 + G) * B], in_=r_t[:])
                    elif rl0 >= XPP:             # all real rows
                        nc_.sync.dma_start(
                            irec_pr[:, rl0 - XPP:rl0 - XPP + G, :], r3)
                    else:                        # straddle
                        k = XPP - rl0
                        nc_.vector.tensor_copy(
                            out=extras_sb[:, rl0 * B:XPP * B], in_=r_t[:, :k * B])
                        nc_.sync.dma_start(
                            irec_pr[:, 0:G - k, :], r3[:, k:, :])
                    if "G" in parts or "R" in parts:
                        continue
                    if (bt + 1) * G >= XPP and scattered < 8 * NCOLS:
                        nb_left = max(NB - bt - 1, 1)
                        per = -(-(8 * NCOLS) // nb_left) if bt + 1 < NB \
                            else 8 * NCOLS - scattered
                        for _ in range(min(per, 8 * NCOLS - scattered)):
                            e = scattered
                            scattered += 1
                            ci, k = divmod(e, 8)
                            ex = k * NCOLS + ci
                            nc_.gpsimd.indirect_dma_start(
                                out=acc_d[k].ap()[:],
                                out_offset=bass.IndirectOffsetOnAxis(
                                    ap=eb_t[:, ex:ex + 1], axis=0),
                                in_=extras_sb[:, ex * B:(ex + 1) * B],
                                in_offset=None, compute_op=OP.add)

                if "s" not in parts:
                    out_sb0 = spool.tile([128, NPP * 72], F32, tag="out0")
                    nc_.vector.memset(out_sb0[:], 0.0)
                    nc_.sync.dma_start(out_t.ap()[:], out_sb0[:])
                    return
                # ---- state phase (loads/precompute hoisted) ----
                irec2 = spool.tile([128, RPP * B], F32, tag="irec2")
                nc_.sync.dma_start(irec2[:], irec_pr)
                acc_sb = spool.tile([128, 200 * B], F32, tag="acc_sb")
                for k in range(8):
                    nc_.sync.dma_start(
                        acc_sb[:, k * 25 * B:(k + 1) * 25 * B]
                            .rearrange("p (u b) -> p u b", b=B),
                        acc_d[k].ap()[:128 * 25].rearrange("(p u) b -> p u b", p=128))
                nc_.vector.tensor_tensor(out=irec2[:], in0=irec2[:],
                                         in1=acc_sb[:, :RPP * B], op=OP.add)

                def load(name, ap, sz):
                    t = spool.tile([128, sz], F32, tag=name)
                    nc_.sync.dma_start(t[:], ap)
                    return t

                tin = load("inputs", inputs_l, RPP * B)
                tpr = load("psc_rise", psc_rise_l, RPP * B)
                tps = load("psc", psc_l, RPP * B)
                tz = load("z_slice", z_slice, NPP * D * B)
                tv = load("v", v_l, NPP * B)
                tr = load("r", r_l, NPP * B)
                ta1 = load("asc1", asc1_l, NPP * B)
                ta2 = load("asc2", asc2_l, NPP * B)
                tsd = load("syn_decay", syn_decay_l, RPP)
                tpi = load("psc_initial", psc_initial_l, RPP)
                tk = load("k", k_l, NPP * 2)
                tam = load("asc_amps", asc_amps_l, NPP * 2)
                tp = {k_: load(k_, v_, NPP) for k_, v_ in pn.items()}

                out_sb = spool.tile([128, NPP * 72], F32, tag="out")
                o3 = out_sb[:].rearrange("p (n f) -> p n f", f=72)

                def v4(t):   # [128, RPP*B] tile -> [128, NPP, R, B]
                    return t[:].rearrange("p (n r b) -> p n r b", r=R, b=B)

                def v3(t):   # [128, NPP*B] tile -> [128, NPP, B]
                    return t[:].rearrange("p (n b) -> p n b", b=B)

                def o4(lo, hi):  # out slice [128, NPP, R, B]
                    return o3[:, :, lo:hi].rearrange("p n (r b) -> p n r b", b=B)

                def bc_nr(t):  # [128, RPP] tile -> [128, NPP, R, B] b-broadcast
                    return t[:].rearrange("p (n r) -> p n r", r=R).unsqueeze(3) \
                            .to_broadcast([128, NPP, R, B])

                def bc_n(t):   # [128, NPP] tile -> [128, NPP, B] b-broadcast
                    return t[:].unsqueeze(2).to_broadcast([128, NPP, B])

                tmp = spool.tile([128, RPP * B], F32, tag="tmp")
                tmp2 = spool.tile([128, RPP * B], F32, tag="tmp2")
                tmpn = spool.tile([128, NPP * B], F32, tag="tmpn")
                tmpn2 = spool.tile([128, NPP * B], F32, tag="tmpn2")
                tmpn3 = spool.tile([128, NPP * B], F32, tag="tmpn3")
                tmpn4 = spool.tile([128, NPP * B], F32, tag="tmpn4")
                tpn1 = spool.tile([128, NPP], F32, tag="tpn1")
                tpn2 = spool.tile([128, NPP], F32, tag="tpn2")

                # rec_in = irec + inputs
                nc_.vector.tensor_tensor(out=irec2[:], in0=irec2[:], in1=tin[:], op=OP.add)
                # new_psc_rise = syn_decay*psc_rise + rec_in*psc_initial
                nc_.vector.tensor_tensor(out=v4(tmp), in0=v4(tpr), in1=bc_nr(tsd), op=OP.mult)
                nc_.vector.tensor_tensor(out=v4(tmp2), in0=v4(irec2), in1=bc_nr(tpi), op=OP.mult)
                nc_.vector.tensor_tensor(out=o4(20, 36), in0=v4(tmp), in1=v4(tmp2), op=OP.add)
                # new_psc = syn_decay*(psc + DT*psc_rise)
                nc_.vector.scalar_tensor_tensor(out=tmp[:], in0=tpr[:], scalar=DT,
                                                in1=tps[:], op0=OP.mult, op1=OP.add)
                nc_.vector.tensor_tensor(out=o4(36, 52), in0=v4(tmp), in1=bc_nr(tsd), op=OP.mult)
                # input_current = sum_r psc (old)
                psum_view = tps[:].rearrange("p (n r b) -> p n b r", r=R, b=B)
                nc_.vector.tensor_reduce(out=v3(tmpn), in_=psum_view, axis=AX.X, op=OP.add)
                # prev_z
                pz = tz[:].rearrange("p (n d b) -> p n d b", d=D, b=B)[:, :, 0, :]
                # new_r = relu(r + prev_z*t_ref - DT)   (keep pre-relu copy in tmpn2)
                nc_.vector.tensor_tensor(out=v3(tmpn2), in0=pz, in1=bc_n(tp["t_ref"]), op=OP.mult)
                nc_.vector.tensor_tensor(out=tmpn2[:], in0=tmpn2[:], in1=tr[:], op=OP.add)
                nc_.vector.tensor_scalar(out=tmpn2[:], in0=tmpn2[:], scalar1=-DT,
                                         scalar2=None, op0=OP.add)
                nc_.scalar.activation(out=tmpn2[:], in_=tmpn2[:], func=ACT.Relu)
                nc_.vector.tensor_copy(out=o3[:, :, 8:12], in_=v3(tmpn2))
                # e_i = exp(-DT*sigmoid(k))
                nc_.scalar.activation(out=tk[:], in_=tk[:], func=ACT.Sigmoid)
                nc_.vector.tensor_scalar(out=tk[:], in0=tk[:], scalar1=-DT,
                                         scalar2=None, op0=OP.mult)
                nc_.scalar.activation(out=tk[:], in_=tk[:], func=ACT.Exp)
                k2 = tk[:].rearrange("p (n two) -> p n two", two=2)
                am2 = tam[:].rearrange("p (n two) -> p n two", two=2)
                for idx, (tasc, lo) in enumerate([(ta1, 12), (ta2, 16)]):
                    ei = k2[:, :, idx:idx + 1].to_broadcast([128, NPP, B])
                    ai = am2[:, :, idx:idx + 1].to_broadcast([128, NPP, B])
                    nc_.vector.tensor_tensor(out=v3(tmpn3), in0=v3(tasc), in1=ei, op=OP.mult)
                    nc_.vector.tensor_tensor(out=v3(tmpn4), in0=pz, in1=ai, op=OP.mult)
                    nc_.vector.tensor_tensor(out=o3[:, :, lo:lo + 4], in0=v3(tmpn3),
                                             in1=v3(tmpn4), op=OP.add)
                # c1 = input_current + asc1 + asc2 + g*e_l   (asc old)
                nc_.vector.tensor_tensor(out=tpn1[:], in0=tp["g"][:], in1=tp["e_l"][:], op=OP.mult)
                nc_.vector.tensor_tensor(out=tmpn[:], in0=tmpn[:], in1=ta1[:], op=OP.add)
                nc_.vector.tensor_tensor(out=tmpn[:], in0=tmpn[:], in1=ta2[:], op=OP.add)
                nc_.vector.tensor_tensor(out=v3(tmpn), in0=v3(tmpn), in1=bc_n(tpn1), op=OP.add)
                # reset_current = prev_z*(v_reset - v_th)
                nc_.vector.tensor_tensor(out=tpn2[:], in0=tp["v_reset"][:], in1=tp["v_th"][:],
                                         op=OP.subtract)
                nc_.vector.tensor_tensor(out=v3(tmpn3), in0=pz, in1=bc_n(tpn2), op=OP.mult)
                # new_v = decay*v + current_factor*c1 + reset_current
                nc_.vector.tensor_tensor(out=v3(tmpn), in0=v3(tmpn),
                                         in1=bc_n(tp["current_factor"]), op=OP.mult)
                nc_.vector.tensor_tensor(out=v3(tv), in0=v3(tv), in1=bc_n(tp["decay"]), op=OP.mult)
                nc_.vector.tensor_tensor(out=tmpn[:], in0=tmpn[:], in1=tv[:], op=OP.add)
                nc_.vector.tensor_tensor(out=tmpn[:], in0=tmpn[:], in1=tmpn3[:], op=OP.add)
                # out_v = new_v*vscale + voffset
                nc_.vector.tensor_tensor(out=v3(tmpn3), in0=v3(tmpn),
                                         in1=bc_n(tp["voltage_scale"]), op=OP.mult)
                nc_.vector.tensor_tensor(out=o3[:, :, 4:8], in0=v3(tmpn3),
                                         in1=bc_n(tp["voltage_offset"]), op=OP.add)
                # v_sc = (new_v - v_th) / (v_th - e_l)
                nc_.vector.tensor_tensor(out=tpn1[:], in0=tp["v_th"][:], in1=tp["e_l"][:],
                                         op=OP.subtract)
                nc_.vector.reciprocal(out=tpn1[:], in_=tpn1[:])
                nc_.vector.tensor_tensor(out=v3(tmpn), in0=v3(tmpn), in1=bc_n(tp["v_th"]),
                                         op=OP.subtract)
                nc_.vector.tensor_tensor(out=v3(tmpn), in0=v3(tmpn), in1=bc_n(tpn1), op=OP.mult)
                # new_z = (v_sc > 0) & (new_r <= 0)
                nc_.vector.tensor_scalar(out=tmpn[:], in0=tmpn[:], scalar1=0.0, scalar2=None,
                                         op0=OP.is_gt)
                nc_.vector.tensor_scalar(out=tmpn2[:], in0=tmpn2[:], scalar1=0.0, scalar2=None,
                                         op0=OP.is_le)
                nc_.vector.tensor_tensor(out=tmpn[:], in0=tmpn[:], in1=tmpn2[:], op=OP.mult)
                nc_.vector.tensor_copy(out=o3[:, :, 0:4], in_=v3(tmpn))
                # z_buf out
                nc_.vector.tensor_copy(out=o3[:, :, 52:56], in_=v3(tmpn))
                zsrc = tz[:].rearrange("p (n x) -> p n x", x=D * B)[:, :, 0:(D - 1) * B]
                nc_.vector.tensor_copy(out=o3[:, :, 56:72], in_=zsrc)

                nc_.sync.dma_start(out_t.ap()[:], out_sb[:])

            if reps == 1:
                body()
            else:
                with tc.For_i(0, reps, 1):
                    body()

    nc.compile()
    return nc


# ---------------------------------------------------------------- jax runner

from concourse import bass2jax
from concourse.bass2jax import _bass_exec_p, install_neuronx_cc_hook, partition_id_tensor
from jax.sharding import Mesh, PartitionSpec
from jax.experimental.shard_map import shard_map


def make_runner(nc, n_cores):
    install_neuronx_cc_hook()
    assert nc.dbg_addr is None or not nc.dbg_callbacks
    partition_name = nc.partition_id_tensor.name if nc.partition_id_tensor else None
    in_names, out_names, out_avals, zero_outs = [], [], [], []
    for alloc in nc.m.functions[0].allocations:
        if not isinstance(alloc, mybir.MemoryLocationSet):
            continue
        name = alloc.memorylocations[0].name
        if alloc.kind == "ExternalInput":
            if name != partition_name and (nc.dbg_addr is None or name != nc.dbg_addr.name):
                in_names.append(name)
        elif alloc.kind == "ExternalOutput":
            shape = tuple(alloc.tensor_shape)
            dtype = mybir.dt.np(alloc.dtype)
            out_names.append(name)
            out_avals.append(jax.core.ShapedArray(shape, dtype))
            zero_outs.append(np.zeros(shape, dtype))
    n_params = len(in_names)
    n_outs = len(out_avals)
    in_names_all = list(in_names) + list(out_names)
    if partition_name is not None:
        in_names_all.append(partition_name)

    donate = tuple(range(n_params, n_params + n_outs))

    def _body(*args):
        operands = list(args)
        if partition_name is not None:
            operands.append(partition_id_tensor())
        outs = _bass_exec_p.bind(
            *operands, out_avals=tuple(out_avals), in_names=tuple(in_names_all),
            out_names=tuple(out_names), lowering_input_output_aliases=(),
            sim_require_finite=True, sim_require_nnan=True, nc=nc)
        return tuple(outs)

    if n_cores == 1:
        fn = jax.jit(_body, donate_argnums=donate, keep_unused=True)

        def run(in_map):
            args = [np.asarray(in_map[n]) for n in in_names] + [z.copy() for z in zero_outs]
            outs = fn(*args)
            jax.block_until_ready(outs)
            return {name: np.asarray(outs[i]) for i, name in enumerate(out_names)}
        return run

    devices = jax.devices()[:n_cores]
    mesh = Mesh(np.asarray(devices), ("core",))
    fn = jax.jit(
        shard_map(_body, mesh=mesh, in_specs=(PartitionSpec("core"),) * (n_params + n_outs),
                  out_specs=(PartitionSpec("core"),) * n_outs, check_rep=False),
        donate_argnums=donate, keep_unused=True)

    def run(in_maps):
        concat_in = [np.concatenate([np.asarray(m[n]) for m in in_maps], axis=0) for n in in_names]
        concat_zeros = [np.zeros((n_cores * z.shape[0], *z.shape[1:]), z.dtype) for z in zero_outs]
        outs = fn(*concat_in, *concat_zeros)
        jax.block_until_ready(outs)
        return [
            {name: np.asarray(outs[i]).reshape(n_cores, *out_avals[i].shape)[c]
             for i, name in enumerate(out_names)}
            for c in range(n_cores)
        ]
    return run


_CACHE = {}


def get_program(S, TPP, XPP, NCOLS, reps=1, parts="gs"):
    key = (S, TPP, XPP, NCOLS, reps, parts)
    if key not in _CACHE:
        nc = build_program(S, TPP, XPP, NCOLS, reps=reps, parts=parts)
        _CACHE[key] = make_runner(nc, NCORES)
    return _CACHE[key]


def build_in_maps(inputs):
    zT = np.ascontiguousarray(inputs["z_buf"].T)        # [ND, B]
    nonzero_row = (zT != 0.0).any(axis=1)
    keep = nonzero_row[inputs["rec_cols"]]
    counts = np.bincount(inputs["rec_rows"][keep], minlength=NR)
    S, TPP, XPP, NCOLS = plan_capacity(counts)
    shards = pack_shards(inputs["rec_w"], inputs["rec_rows"].astype(np.int64),
                         inputs["rec_cols"].astype(np.int64), keep, S, TPP, XPP,
                         NCOLS)
    ztab = np.ascontiguousarray(zT.reshape(NBLK, 64))
    in_maps = []
    for c in range(NCORES):
        sh = shards[c]
        sl = relayout_state(inputs, c)
        m = dict(ztab=ztab, idx8=sh["idx8"], res_a=sh["res_a"], w_a=sh["w_a"],
                 extra_base=sh["extra_base"])
        m["inputs_l"] = sl["inputs"].reshape(128, -1)
        m["psc_rise_l"] = sl["psc_rise"].reshape(128, -1)
        m["psc_l"] = sl["psc"].reshape(128, -1)
        m["z_slice"] = sl["z_slice"].reshape(128, -1)
        m["v_l"] = sl["v"].reshape(128, -1)
        m["r_l"] = sl["r"].reshape(128, -1)
        m["asc1_l"] = sl["asc_1"].reshape(128, -1)
        m["asc2_l"] = sl["asc_2"].reshape(128, -1)
        m["syn_decay_l"] = sl["syn_decay"].reshape(128, -1)
        m["psc_initial_l"] = sl["psc_initial"].reshape(128, -1)
        m["k_l"] = sl["k"].reshape(128, -1)
        m["asc_amps_l"] = sl["asc_amps"].reshape(128, -1)
        for name in ["t_ref", "v_th", "e_l", "v_reset", "g", "decay",
                     "current_factor", "voltage_scale", "voltage_offset"]:
            m[name + "_l"] = sl[name].reshape(128, -1)
        in_maps.append(m)
    return in_maps, (S, TPP, XPP, NCOLS)


def kernel(**inputs) -> np.ndarray:
    inputs = {k: np.asarray(v) for k, v in inputs.items()}
    in_maps, (S, TPP, XPP, NCOLS) = build_in_maps(inputs)
    run = get_program(S, TPP, XPP, NCOLS, reps=1)
    results = run(in_maps)
    core_outs = [results[c]["out"].reshape(128, NPP, 72) for c in range(NCORES)]
    return assemble_output(core_outs)
